# revision 16
# baseline (speedup 1.0000x reference)
"""Trainium2 Bass kernel for nn_CosineLayer (retrieval_knn).

Computes out = concat(normalize(features) @ normalize(weight).T, threshold_col).

Strategy (tensor/vocab parallel on the 434k concept axis, per sharding hint):
  - Host: L2-normalize features and weight rows, fold normalization into the
    weight, quantize the weight to fp8 e3m4 (x32 global scale; measured
    1.3e-2 entry rms rel err on this data -> 1.139e-2 output rel_l2 vs the
    2e-2 gate), and pre-swizzle each shard to [128, chunk, kc, nt] so every
    per-partition DMA line is kc*nt = 6KB contiguous.
  - Device (x8 SPMD): streaming matmul sim_shard = f_hatT.T @ w_q_shard with
    fp16 stationary features x e3m4 moving weights (mixed-dtype matmul is
    exact on TRN2; probed), fp32 PSUM accumulation over K=768 in 6 chunks of
    128. The e3m4 weight stream halves HBM traffic vs fp16 (41.7MB + 27.8MB
    out vs 111MB per core) and pins the kernel at the PE roofline (271us: 1
    moving column/cycle at 2.4GHz; fp8e4 DoubleRow would be 2x but its 2.7e-2
    quantization error fails the gate — probed + measured). Startup is hidden
    by 14 warmup matmuls on a zeroed tile (PE p-state ramps during the first
    DMA wait) and by splitting the first chunk into 4 pieces; the last chunk
    is split too so the output tail drains early. Measured 294us on a
    full-clock run (vs 326us fp16 baseline); chip DVFS adds run-to-run
    variance of up to ~20%.
  - Host: concat shard outputs, un-scale (/32), trim padding, append
    threshold column. A per-core 128-column probe is checked against the
    host and the execute is retried on mismatch (a flaky device execute was
    observed to silently return garbage once).
"""

import os

import numpy as np
import ml_dtypes

import concourse.mybir as mybir
import concourse.tile as tile
from concourse import bacc
from concourse.bass_utils import run_bass_kernel_spmd

N_CORES = 8
B = 256              # feature rows
K = 768              # embedding dim
KC = K // 128        # 6 k-chunks of 128 partitions
N_FULL = 434056      # concept rows
N_SHARD = 54272      # = 106*512; 8*54272 = 434176 (pad 120)
NT = int(os.environ.get("BASS_COSINE_NT", "1024"))   # n-columns per chunk
N_CHUNKS = N_SHARD // NT
OUT_BATCH = int(os.environ.get("BASS_COSINE_OUT_BATCH", "1"))  # chunks per out-DMA
EPS = 1e-8

# weight compute dtype. "e3" = fp8 e3m4 weights (x32 scale, fp16 features,
# fp16 x32-scaled output): halves weight HBM traffic vs fp16; measured
# ~1.1e-2 rel_l2 vs the fp32 reference on the seed-0 data (gate 2e-2).
# "e3s" = same but features also e3m4 (fallback if mixed-dtype matmul
# misbehaves): ~1.6e-2 rel_l2.
MODE = os.environ.get("BASS_COSINE_MODE", "e3")
W_SCALE = 32.0

_CACHED = {}


def _build_bass(mode):
    """Build + compile the single-core program (same NEFF runs on all 8 cores)."""
    assert mode in ("e3", "e3s", "fp16x")
    assert N_CHUNKS % OUT_BATCH == 0, "OUT_BATCH must divide N_CHUNKS"
    nc = bacc.Bacc("TRN2", target_bir_lowering=False, debug=False,
                   num_devices=N_CORES)
    wdt = mybir.dt.float16 if mode == "fp16x" else mybir.dt.float8e3
    fdt = mybir.dt.float8e3 if mode == "e3s" else mybir.dt.float16
    fT_d = nc.dram_tensor("fT", [K, B], fdt, kind="ExternalInput").ap()
    # pre-swizzled so chunk g is [128, KC, NT] with KC*NT contiguous per row
    wT_d = nc.dram_tensor("wT", [128, N_CHUNKS, KC, NT], wdt,
                          kind="ExternalInput").ap()
    odt = mybir.dt.float16
    out_d = nc.dram_tensor("out", [B, N_SHARD], odt, kind="ExternalOutput").ap()

    fT_r = fT_d.rearrange("(c p) b -> p c b", p=128)   # [128, KC, B]

    n_warm = int(os.environ.get("BASS_COSINE_WARMUP", "12"))
    first_split = int(os.environ.get("BASS_COSINE_FIRST_SPLIT", "4"))
    assert NT % first_split == 0 and NT // first_split >= 128

    with tile.TileContext(nc) as tc:
        with (
            tc.tile_pool(name="fpool", bufs=1) as fpool,
            tc.tile_pool(name="wpool", bufs=4) as wpool,
            tc.tile_pool(name="opool", bufs=3) as opool,
            tc.tile_pool(name="psum", bufs=4, space="PSUM") as psum,
        ):
            # chunk 0 split into small pieces so the first matmul's data
            # lands ASAP; warmup matmuls on a zeroed tile ramp the PE
            # p-state out of the DMA-wait shadow.
            fnt = NT // first_split
            pieces = [(j * fnt, fnt) for j in range(first_split)]
            pieces += [(n * NT, NT) for n in range(1, N_CHUNKS - 1)]
            # last chunk split too: its output drains per piece, shrinking
            # the end-of-kernel CAST+DMA tail
            last0 = (N_CHUNKS - 1) * NT
            pieces += [(last0 + j * fnt, fnt) for j in range(first_split)]

            wsbs = {}
            wsbs[0] = wpool.tile([128, KC, fnt], wdt, name="wsb_f0",
                                 tag="wsb_first")
            nc.sync.dma_start(wsbs[0][:], wT_d[:, 0, :, 0:fnt])

            # per-c feature DMAs: with subtile deps the first matmul only
            # waits on the c=0 slice, not the whole feature upload
            fsb = fpool.tile([128, KC, B], fdt)
            for c in range(KC):
                nc.sync.dma_start(fsb[:, c], fT_r[:, c])

            if n_warm:
                wu = fpool.tile([128, 512], mybir.dt.float16, name="warm",
                                tag="warm")
                nc.any.memset(wu, 0.0)
                pwu = psum.tile([128, 512], mybir.dt.float32, name="psw",
                                tag="ps0")
                for _ in range(n_warm):
                    nc.tensor.matmul(pwu[:], wu[:, 0:128], wu[:],
                                     start=True, stop=True)

            for pi, (n0, nt) in enumerate(pieces):
                if pi not in wsbs:
                    g0 = n0 // NT
                    t0 = n0 - g0 * NT
                    tag = "wsb_first" if nt != NT else "wsbN"
                    wsbs[pi] = wpool.tile([128, KC, nt], wdt,
                                          name=f"wsb{pi}", tag=tag)
                    nc.sync.dma_start(wsbs[pi][:],
                                      wT_d[:, g0, :, t0:t0 + nt])
                if pi + 1 < len(pieces) and (pi + 1) not in wsbs:
                    n0n, ntn = pieces[pi + 1]
                    g0 = n0n // NT
                    t0 = n0n - g0 * NT
                    tag = "wsb_first" if ntn != NT else "wsbN"
                    wsbs[pi + 1] = wpool.tile([128, KC, ntn], wdt,
                                              name=f"wsb{pi + 1}", tag=tag)
                    nc.sync.dma_start(wsbs[pi + 1][:],
                                      wT_d[:, g0, :, t0:t0 + ntn])

                g = n0 // NT
                j0 = n0 - g * NT
                if j0 == 0:
                    osb = [
                        opool.tile([128, NT], odt, name=f"osb{b}", tag=f"osb{b}")
                        for b in range(B // 128)
                    ]
                wsb = wsbs.pop(pi)
                nh = max(1, nt // 512)
                hs = min(nt, 512)
                for b in range(B // 128):
                    # h innermost so both h-slices share one LDWEIGHTS
                    # per (b, c) stationary f-tile
                    pss = [
                        psum.tile([128, 512], mybir.dt.float32,
                                  name=f"ps{h}", tag=f"ps{h}")
                        for h in range(nh)
                    ]
                    for c in range(KC):
                        for h in range(nh):
                            nc.tensor.matmul(
                                pss[h][:, 0:hs],
                                fsb[:, c, b * 128:(b + 1) * 128],
                                wsb[:, c, h * hs:(h + 1) * hs],
                                start=(c == 0),
                                stop=(c == KC - 1),
                            )
                    for h in range(nh):
                        nc.vector.tensor_copy(
                            osb[b][:, j0 + h * hs: j0 + (h + 1) * hs],
                            pss[h][:, 0:hs],
                        )
                # output DMAs ride the ACT/DVE HWDGE rings (alternating per
                # chunk) so they don't queue behind the next chunk's input
                # DMA on SP, nor behind each other at the drain
                oring = nc.scalar if g % 2 == 0 else nc.gpsimd
                if g == N_CHUNKS - 1:
                    for b in range(B // 128):
                        (nc.scalar if (j0 // fnt) % 2 == 0 else nc.gpsimd).dma_start(
                            out_d[b * 128:(b + 1) * 128, n0:n0 + nt],
                            osb[b][:, j0:j0 + nt]
                        )
                elif j0 + nt == NT:
                    for b in range(B // 128):
                        oring.dma_start(
                            out_d[b * 128:(b + 1) * 128, g * NT:(g + 1) * NT],
                            osb[b][:]
                        )
    nc.compile()
    return nc


def _run_spmd(nc, in_maps):
    last_exc = None
    for _ in range(3):  # device occasionally needs one recovery execute
        try:
            return run_bass_kernel_spmd(nc, in_maps, core_ids=list(range(N_CORES)))
        except Exception as e:  # noqa: BLE001
            last_exc = e
    raise last_exc


def _shards_ok(res, f_hat, weight, w_inv, inv_scale):
    """Guard against flaky device executes (observed: a run can silently
    return decorrelated garbage). Check a 128-column probe block per core
    against the host; caller reruns on failure."""
    ok = True
    for i in range(N_CORES):
        n0 = i * N_SHARD
        w_hat_blk = weight[n0:n0 + 128] * (w_inv[n0:n0 + 128] / W_SCALE)
        ref = f_hat @ w_hat_blk.T                   # [B, 128] fp32
        got = res.results[i]["out"][:, :128].astype(np.float32) * inv_scale
        err = np.abs(got - ref).max()
        if not np.isfinite(err) or err > 0.05:
            print(f"kernel self-check: core {i} probe absmax {err:.3e} "
                  f"-> rerun", flush=True)
            ok = False
    return ok


def _swizzle_shard(wq):
    """[N_SHARD, K] row-major -> [128, N_CHUNKS, KC, NT] so each chunk's
    per-partition line (KC*NT bytes) is contiguous."""
    # buf[p, g, c, t] = wq[g*NT + t, c*128 + p]
    v = wq.reshape(N_CHUNKS, NT, KC, 128)
    return np.ascontiguousarray(v.transpose(3, 0, 2, 1))


def kernel(features, weight, threshold):
    features = np.asarray(features, dtype=np.float32)
    weight = np.asarray(weight, dtype=np.float32)

    f_norm = np.linalg.norm(features, axis=1, keepdims=True)
    f_hat = features / np.maximum(f_norm, EPS)
    if MODE == "e3s":
        fT = np.ascontiguousarray(f_hat.T * W_SCALE).astype(ml_dtypes.float8_e3m4)
    else:
        fT = np.ascontiguousarray(f_hat.T).astype(np.float16)  # [768, 256]

    w_norm = np.linalg.norm(weight, axis=1, keepdims=True)
    w_inv = (W_SCALE / np.maximum(w_norm, EPS)).astype(np.float32)
    wnp = np.float16 if MODE == "fp16x" else ml_dtypes.float8_e3m4

    shards = []
    for i in range(N_CORES):
        n0 = i * N_SHARD
        n1 = min(n0 + N_SHARD, N_FULL)
        s = np.zeros((N_SHARD, K), dtype=wnp)
        s[: n1 - n0] = (weight[n0:n1] * w_inv[n0:n1]).astype(wnp)
        shards.append(_swizzle_shard(s))

    key = ("nc", MODE)
    if key not in _CACHED:
        _CACHED[key] = _build_bass(MODE)
    nc = _CACHED[key]

    inv_scale = np.float32(1.0 / W_SCALE)
    if MODE == "e3s":
        inv_scale = np.float32(1.0 / (W_SCALE * W_SCALE))

    in_maps = [{"fT": fT, "wT": shards[i]} for i in range(N_CORES)]
    res = _run_spmd(nc, in_maps)
    for _ in range(3):
        if _shards_ok(res, f_hat, weight, w_inv, inv_scale):
            break
        res = _run_spmd(nc, in_maps)
    _CACHED["last_result"] = res
    out = np.empty((B, N_FULL + 1), dtype=np.float32)
    for i in range(N_CORES):
        n0 = i * N_SHARD
        n1 = min(n0 + N_SHARD, N_FULL)
        out[:, n0:n1] = res.results[i]["out"][:, : n1 - n0].astype(np.float32)
        out[:, n0:n1] *= inv_scale
    out[:, N_FULL] = np.float32(threshold)
    return out


# revision 17
# speedup vs baseline: 1.0032x; 1.0032x over previous
"""Trainium2 Bass kernel for nn_CosineLayer (retrieval_knn).

Computes out = concat(normalize(features) @ normalize(weight).T, threshold_col).

Strategy (tensor/vocab parallel on the 434k concept axis, per sharding hint):
  - Host: L2-normalize features and weight rows, fold normalization into the
    weight, quantize the weight to fp8 e3m4 (x32 global scale; measured
    1.3e-2 entry rms rel err on this data -> 1.139e-2 output rel_l2 vs the
    2e-2 gate), and pre-swizzle each shard to [128, chunk, kc, nt] so every
    per-partition DMA line is kc*nt = 6KB contiguous.
  - Device (x8 SPMD): streaming matmul sim_shard = f_hatT.T @ w_q_shard with
    fp16 stationary features x e3m4 moving weights (mixed-dtype matmul is
    exact on TRN2; probed), fp32 PSUM accumulation over K=768 in 6 chunks of
    128. The e3m4 weight stream halves HBM traffic vs fp16 (41.7MB + 27.8MB
    out vs 111MB per core) and pins the kernel at the PE roofline (271us: 1
    moving column/cycle at 2.4GHz; fp8e4 DoubleRow would be 2x but its 2.7e-2
    quantization error fails the gate — probed + measured). Startup is hidden
    by 14 warmup matmuls on a zeroed tile (PE p-state ramps during the first
    DMA wait) and by splitting the first chunk into 4 pieces; the last chunk
    is split too so the output tail drains early. Measured 294us on a
    full-clock run (vs 326us fp16 baseline); chip DVFS adds run-to-run
    variance of up to ~20%.
  - Host: concat shard outputs, un-scale (/32), trim padding, append
    threshold column. A per-core 128-column probe is checked against the
    host and the execute is retried on mismatch (a flaky device execute was
    observed to silently return garbage once).
"""

import os

import numpy as np
import ml_dtypes

import concourse.mybir as mybir
import concourse.tile as tile
from concourse import bacc
from concourse.bass_utils import run_bass_kernel_spmd

N_CORES = 8
B = 256              # feature rows
K = 768              # embedding dim
KC = K // 128        # 6 k-chunks of 128 partitions
N_FULL = 434056      # concept rows
N_SHARD = 54272      # = 106*512; 8*54272 = 434176 (pad 120)
NT = int(os.environ.get("BASS_COSINE_NT", "1024"))   # n-columns per chunk
N_CHUNKS = N_SHARD // NT
OUT_BATCH = int(os.environ.get("BASS_COSINE_OUT_BATCH", "1"))  # chunks per out-DMA
EPS = 1e-8

# weight compute dtype. "e3" = fp8 e3m4 weights (x32 scale, fp16 features,
# fp16 x32-scaled output): halves weight HBM traffic vs fp16; measured
# ~1.1e-2 rel_l2 vs the fp32 reference on the seed-0 data (gate 2e-2).
# "e3s" = same but features also e3m4 (fallback if mixed-dtype matmul
# misbehaves): ~1.6e-2 rel_l2.
MODE = os.environ.get("BASS_COSINE_MODE", "e3")
W_SCALE = 32.0

_CACHED = {}


def _build_bass(mode):
    """Build + compile the single-core program (same NEFF runs on all 8 cores)."""
    assert mode in ("e3", "e3s", "fp16x")
    assert N_CHUNKS % OUT_BATCH == 0, "OUT_BATCH must divide N_CHUNKS"
    nc = bacc.Bacc("TRN2", target_bir_lowering=False, debug=False,
                   num_devices=N_CORES)
    wdt = mybir.dt.float16 if mode == "fp16x" else mybir.dt.float8e3
    fdt = mybir.dt.float8e3 if mode == "e3s" else mybir.dt.float16
    fT_d = nc.dram_tensor("fT", [K, B], fdt, kind="ExternalInput").ap()
    # pre-swizzled so chunk g is [128, KC, NT] with KC*NT contiguous per row
    wT_d = nc.dram_tensor("wT", [128, N_CHUNKS, KC, NT], wdt,
                          kind="ExternalInput").ap()
    odt = mybir.dt.float16
    out_d = nc.dram_tensor("out", [B, N_SHARD], odt, kind="ExternalOutput").ap()

    fT_r = fT_d.rearrange("(c p) b -> p c b", p=128)   # [128, KC, B]

    n_warm = int(os.environ.get("BASS_COSINE_WARMUP", "12"))
    first_split = int(os.environ.get("BASS_COSINE_FIRST_SPLIT", "4"))
    assert NT % first_split == 0 and NT // first_split >= 128

    with tile.TileContext(nc) as tc:
        with (
            tc.tile_pool(name="fpool", bufs=1) as fpool,
            tc.tile_pool(name="wpool", bufs=4) as wpool,
            tc.tile_pool(name="opool", bufs=3) as opool,
            tc.tile_pool(name="psum", bufs=4, space="PSUM") as psum,
        ):
            # chunk 0 split into small pieces so the first matmul's data
            # lands ASAP; warmup matmuls on a zeroed tile ramp the PE
            # p-state out of the DMA-wait shadow.
            fnt = NT // first_split
            pieces = [(j * fnt, fnt) for j in range(first_split)]
            pieces += [(n * NT, NT) for n in range(1, N_CHUNKS - 1)]
            # last chunk split too: its output drains per piece, shrinking
            # the end-of-kernel CAST+DMA tail
            last0 = (N_CHUNKS - 1) * NT
            pieces += [(last0 + j * fnt, fnt) for j in range(first_split)]

            wsbs = {}
            for j in (0, 1):  # pieces 0+1 up front so piece 1 isn't queued
                wsbs[j] = wpool.tile([128, KC, fnt], wdt, name=f"wsb_f{j}",
                                     tag="wsb_first")
                nc.sync.dma_start(wsbs[j][:],
                                  wT_d[:, 0, :, j * fnt:(j + 1) * fnt])

            # per-c feature DMAs: with subtile deps the first matmul only
            # waits on the c=0 slice, not the whole feature upload
            fsb = fpool.tile([128, KC, B], fdt)
            for c in range(KC):
                nc.sync.dma_start(fsb[:, c], fT_r[:, c])

            if n_warm:
                wu = fpool.tile([128, 512], mybir.dt.float16, name="warm",
                                tag="warm")
                nc.any.memset(wu, 0.0)
                pwu = psum.tile([128, 512], mybir.dt.float32, name="psw",
                                tag="ps0")
                for _ in range(n_warm):
                    nc.tensor.matmul(pwu[:], wu[:, 0:128], wu[:],
                                     start=True, stop=True)

            for pi, (n0, nt) in enumerate(pieces):
                if pi not in wsbs:
                    g0 = n0 // NT
                    t0 = n0 - g0 * NT
                    tag = "wsb_first" if nt != NT else "wsbN"
                    wsbs[pi] = wpool.tile([128, KC, nt], wdt,
                                          name=f"wsb{pi}", tag=tag)
                    nc.sync.dma_start(wsbs[pi][:],
                                      wT_d[:, g0, :, t0:t0 + nt])
                if pi + 1 < len(pieces) and (pi + 1) not in wsbs:
                    n0n, ntn = pieces[pi + 1]
                    g0 = n0n // NT
                    t0 = n0n - g0 * NT
                    tag = "wsb_first" if ntn != NT else "wsbN"
                    wsbs[pi + 1] = wpool.tile([128, KC, ntn], wdt,
                                              name=f"wsb{pi + 1}", tag=tag)
                    nc.sync.dma_start(wsbs[pi + 1][:],
                                      wT_d[:, g0, :, t0:t0 + ntn])

                g = n0 // NT
                j0 = n0 - g * NT
                if j0 == 0:
                    osb = [
                        opool.tile([128, NT], odt, name=f"osb{b}", tag=f"osb{b}")
                        for b in range(B // 128)
                    ]
                wsb = wsbs.pop(pi)
                nh = max(1, nt // 512)
                hs = min(nt, 512)
                for b in range(B // 128):
                    # h innermost so both h-slices share one LDWEIGHTS
                    # per (b, c) stationary f-tile
                    pss = [
                        psum.tile([128, 512], mybir.dt.float32,
                                  name=f"ps{h}", tag=f"ps{h}")
                        for h in range(nh)
                    ]
                    for c in range(KC):
                        for h in range(nh):
                            nc.tensor.matmul(
                                pss[h][:, 0:hs],
                                fsb[:, c, b * 128:(b + 1) * 128],
                                wsb[:, c, h * hs:(h + 1) * hs],
                                start=(c == 0),
                                stop=(c == KC - 1),
                            )
                    for h in range(nh):
                        nc.vector.tensor_copy(
                            osb[b][:, j0 + h * hs: j0 + (h + 1) * hs],
                            pss[h][:, 0:hs],
                        )
                # output DMAs ride the ACT/DVE HWDGE rings (alternating per
                # chunk) so they don't queue behind the next chunk's input
                # DMA on SP, nor behind each other at the drain
                oring = nc.scalar if g % 2 == 0 else nc.gpsimd
                if g == N_CHUNKS - 1:
                    for b in range(B // 128):
                        (nc.scalar if (j0 // fnt) % 2 == 0 else nc.gpsimd).dma_start(
                            out_d[b * 128:(b + 1) * 128, n0:n0 + nt],
                            osb[b][:, j0:j0 + nt]
                        )
                elif j0 + nt == NT:
                    for b in range(B // 128):
                        oring.dma_start(
                            out_d[b * 128:(b + 1) * 128, g * NT:(g + 1) * NT],
                            osb[b][:]
                        )
    nc.compile()
    return nc


def _run_spmd(nc, in_maps):
    last_exc = None
    for _ in range(3):  # device occasionally needs one recovery execute
        try:
            return run_bass_kernel_spmd(nc, in_maps, core_ids=list(range(N_CORES)))
        except Exception as e:  # noqa: BLE001
            last_exc = e
    raise last_exc


def _shards_ok(res, f_hat, weight, w_inv, inv_scale):
    """Guard against flaky device executes (observed: a run can silently
    return decorrelated garbage). Check a 128-column probe block per core
    against the host; caller reruns on failure."""
    ok = True
    for i in range(N_CORES):
        n0 = i * N_SHARD
        w_hat_blk = weight[n0:n0 + 128] * (w_inv[n0:n0 + 128] / W_SCALE)
        ref = f_hat @ w_hat_blk.T                   # [B, 128] fp32
        got = res.results[i]["out"][:, :128].astype(np.float32) * inv_scale
        err = np.abs(got - ref).max()
        if not np.isfinite(err) or err > 0.05:
            print(f"kernel self-check: core {i} probe absmax {err:.3e} "
                  f"-> rerun", flush=True)
            ok = False
    return ok


def _swizzle_shard(wq):
    """[N_SHARD, K] row-major -> [128, N_CHUNKS, KC, NT] so each chunk's
    per-partition line (KC*NT bytes) is contiguous."""
    # buf[p, g, c, t] = wq[g*NT + t, c*128 + p]
    v = wq.reshape(N_CHUNKS, NT, KC, 128)
    return np.ascontiguousarray(v.transpose(3, 0, 2, 1))


def kernel(features, weight, threshold):
    features = np.asarray(features, dtype=np.float32)
    weight = np.asarray(weight, dtype=np.float32)

    f_norm = np.linalg.norm(features, axis=1, keepdims=True)
    f_hat = features / np.maximum(f_norm, EPS)
    if MODE == "e3s":
        fT = np.ascontiguousarray(f_hat.T * W_SCALE).astype(ml_dtypes.float8_e3m4)
    else:
        fT = np.ascontiguousarray(f_hat.T).astype(np.float16)  # [768, 256]

    w_norm = np.linalg.norm(weight, axis=1, keepdims=True)
    w_inv = (W_SCALE / np.maximum(w_norm, EPS)).astype(np.float32)
    wnp = np.float16 if MODE == "fp16x" else ml_dtypes.float8_e3m4

    shards = []
    for i in range(N_CORES):
        n0 = i * N_SHARD
        n1 = min(n0 + N_SHARD, N_FULL)
        s = np.zeros((N_SHARD, K), dtype=wnp)
        s[: n1 - n0] = (weight[n0:n1] * w_inv[n0:n1]).astype(wnp)
        shards.append(_swizzle_shard(s))

    key = ("nc", MODE)
    if key not in _CACHED:
        _CACHED[key] = _build_bass(MODE)
    nc = _CACHED[key]

    inv_scale = np.float32(1.0 / W_SCALE)
    if MODE == "e3s":
        inv_scale = np.float32(1.0 / (W_SCALE * W_SCALE))

    in_maps = [{"fT": fT, "wT": shards[i]} for i in range(N_CORES)]
    res = _run_spmd(nc, in_maps)
    for _ in range(3):
        if _shards_ok(res, f_hat, weight, w_inv, inv_scale):
            break
        res = _run_spmd(nc, in_maps)
    _CACHED["last_result"] = res
    out = np.empty((B, N_FULL + 1), dtype=np.float32)
    for i in range(N_CORES):
        n0 = i * N_SHARD
        n1 = min(n0 + N_SHARD, N_FULL)
        out[:, n0:n1] = res.results[i]["out"][:, : n1 - n0].astype(np.float32)
        out[:, n0:n1] *= inv_scale
    out[:, N_FULL] = np.float32(threshold)
    return out


# revision 18
# speedup vs baseline: 1.0160x; 1.0128x over previous
"""Trainium2 Bass kernel for nn_CosineLayer (retrieval_knn).

Computes out = concat(normalize(features) @ normalize(weight).T, threshold_col).

Strategy (tensor/vocab parallel on the 434k concept axis, per sharding hint):
  - Host: L2-normalize features and weight rows, fold normalization into the
    weight, quantize the weight to fp8 e3m4 (x32 global scale; measured
    1.3e-2 entry rms rel err on this data -> 1.139e-2 output rel_l2 vs the
    2e-2 gate), and pre-swizzle each shard to [128, chunk, kc, nt] so every
    per-partition DMA line is kc*nt = 6KB contiguous.
  - Device (x8 SPMD): streaming matmul sim_shard = f_hatT.T @ w_q_shard with
    fp16 stationary features x e3m4 moving weights (mixed-dtype matmul is
    exact on TRN2; probed), fp32 PSUM accumulation over K=768 in 6 chunks of
    128. The e3m4 weight stream halves HBM traffic vs fp16 (41.7MB + 27.8MB
    out vs 111MB per core) and pins the kernel at the PE roofline (271us: 1
    moving column/cycle at 2.4GHz; fp8e4 DoubleRow would be 2x but its 2.7e-2
    quantization error fails the gate — probed + measured). Startup is hidden
    by 14 warmup matmuls on a zeroed tile (PE p-state ramps during the first
    DMA wait) and by splitting the first chunk into 4 pieces; the last chunk
    is split too so the output tail drains early. Measured 294us on a
    full-clock run (vs 326us fp16 baseline); chip DVFS adds run-to-run
    variance of up to ~20%.
  - Host: concat shard outputs, un-scale (/32), trim padding, append
    threshold column. A per-core 128-column probe is checked against the
    host and the execute is retried on mismatch (a flaky device execute was
    observed to silently return garbage once).
"""

import os

import numpy as np
import ml_dtypes

import concourse.mybir as mybir
import concourse.tile as tile
from concourse import bacc
from concourse.bass_utils import run_bass_kernel_spmd

N_CORES = 8
B = 256              # feature rows
K = 768              # embedding dim
KC = K // 128        # 6 k-chunks of 128 partitions
N_FULL = 434056      # concept rows
N_SHARD = 54272      # = 106*512; 8*54272 = 434176 (pad 120)
NT = int(os.environ.get("BASS_COSINE_NT", "1024"))   # n-columns per chunk
N_CHUNKS = N_SHARD // NT
OUT_BATCH = int(os.environ.get("BASS_COSINE_OUT_BATCH", "1"))  # chunks per out-DMA
EPS = 1e-8

# weight compute dtype. "e3" = fp8 e3m4 weights (x32 scale, fp16 features,
# fp16 x32-scaled output): halves weight HBM traffic vs fp16; measured
# ~1.1e-2 rel_l2 vs the fp32 reference on the seed-0 data (gate 2e-2).
# "e3s" = same but features also e3m4 (fallback if mixed-dtype matmul
# misbehaves): ~1.6e-2 rel_l2.
MODE = os.environ.get("BASS_COSINE_MODE", "e3")
W_SCALE = 32.0

_CACHED = {}


def _build_bass(mode):
    """Build + compile the single-core program (same NEFF runs on all 8 cores)."""
    assert mode in ("e3", "e3s", "fp16x")
    assert N_CHUNKS % OUT_BATCH == 0, "OUT_BATCH must divide N_CHUNKS"
    nc = bacc.Bacc("TRN2", target_bir_lowering=False, debug=False,
                   num_devices=N_CORES)
    wdt = mybir.dt.float16 if mode == "fp16x" else mybir.dt.float8e3
    fdt = mybir.dt.float8e3 if mode == "e3s" else mybir.dt.float16
    fT_d = nc.dram_tensor("fT", [K, B], fdt, kind="ExternalInput").ap()
    # pre-swizzled so chunk g is [128, KC, NT] with KC*NT contiguous per row
    wT_d = nc.dram_tensor("wT", [128, N_CHUNKS, KC, NT], wdt,
                          kind="ExternalInput").ap()
    odt = mybir.dt.float16
    out_d = nc.dram_tensor("out", [B, N_SHARD], odt, kind="ExternalOutput").ap()

    fT_r = fT_d.rearrange("(c p) b -> p c b", p=128)   # [128, KC, B]

    n_warm = int(os.environ.get("BASS_COSINE_WARMUP", "14"))
    first_split = int(os.environ.get("BASS_COSINE_FIRST_SPLIT", "4"))
    assert NT % first_split == 0 and NT // first_split >= 128

    with tile.TileContext(nc) as tc:
        with (
            tc.tile_pool(name="fpool", bufs=1) as fpool,
            tc.tile_pool(name="wpool", bufs=4) as wpool,
            tc.tile_pool(name="opool", bufs=3) as opool,
            tc.tile_pool(name="psum", bufs=4, space="PSUM") as psum,
        ):
            # chunk 0 split into small pieces so the first matmul's data
            # lands ASAP; warmup matmuls on a zeroed tile ramp the PE
            # p-state out of the DMA-wait shadow.
            fnt = NT // first_split
            pieces = [(j * fnt, fnt) for j in range(first_split)]
            pieces += [(n * NT, NT) for n in range(1, N_CHUNKS - 1)]
            # last chunk split too: its output drains per piece, shrinking
            # the end-of-kernel CAST+DMA tail
            last0 = (N_CHUNKS - 1) * NT
            pieces += [(last0 + j * fnt, fnt) for j in range(first_split)]

            wsbs = {}
            wsbs[0] = wpool.tile([128, KC, fnt], wdt, name="wsb_f0",
                                 tag="wsb_first")
            nc.sync.dma_start(wsbs[0][:], wT_d[:, 0, :, 0:fnt])

            fsb = fpool.tile([128, KC, B], fdt)
            nc.sync.dma_start(fsb[:], fT_r[:])

            if n_warm:
                wu = fpool.tile([128, 512], mybir.dt.float16, name="warm",
                                tag="warm")
                nc.any.memset(wu, 0.0)
                pwu = psum.tile([128, 512], mybir.dt.float32, name="psw",
                                tag="ps0")
                for _ in range(n_warm):
                    nc.tensor.matmul(pwu[:], wu[:, 0:128], wu[:],
                                     start=True, stop=True)

            for pi, (n0, nt) in enumerate(pieces):
                if pi not in wsbs:
                    g0 = n0 // NT
                    t0 = n0 - g0 * NT
                    tag = "wsb_first" if nt != NT else "wsbN"
                    wsbs[pi] = wpool.tile([128, KC, nt], wdt,
                                          name=f"wsb{pi}", tag=tag)
                    nc.sync.dma_start(wsbs[pi][:],
                                      wT_d[:, g0, :, t0:t0 + nt])
                if pi + 1 < len(pieces) and (pi + 1) not in wsbs:
                    n0n, ntn = pieces[pi + 1]
                    g0 = n0n // NT
                    t0 = n0n - g0 * NT
                    tag = "wsb_first" if ntn != NT else "wsbN"
                    wsbs[pi + 1] = wpool.tile([128, KC, ntn], wdt,
                                              name=f"wsb{pi + 1}", tag=tag)
                    nc.sync.dma_start(wsbs[pi + 1][:],
                                      wT_d[:, g0, :, t0:t0 + ntn])

                g = n0 // NT
                j0 = n0 - g * NT
                if j0 == 0:
                    osb = [
                        opool.tile([128, NT], odt, name=f"osb{b}", tag=f"osb{b}")
                        for b in range(B // 128)
                    ]
                wsb = wsbs.pop(pi)
                nh = max(1, nt // 512)
                hs = min(nt, 512)
                for b in range(B // 128):
                    # h innermost so both h-slices share one LDWEIGHTS
                    # per (b, c) stationary f-tile
                    pss = [
                        psum.tile([128, 512], mybir.dt.float32,
                                  name=f"ps{h}", tag=f"ps{h}")
                        for h in range(nh)
                    ]
                    for c in range(KC):
                        for h in range(nh):
                            nc.tensor.matmul(
                                pss[h][:, 0:hs],
                                fsb[:, c, b * 128:(b + 1) * 128],
                                wsb[:, c, h * hs:(h + 1) * hs],
                                start=(c == 0),
                                stop=(c == KC - 1),
                            )
                    for h in range(nh):
                        nc.vector.tensor_copy(
                            osb[b][:, j0 + h * hs: j0 + (h + 1) * hs],
                            pss[h][:, 0:hs],
                        )
                # output DMAs ride the ACT HWDGE ring so they don't
                # queue behind the next chunk's input DMA on SP
                if g == N_CHUNKS - 1:
                    for b in range(B // 128):
                        nc.scalar.dma_start(
                            out_d[b * 128:(b + 1) * 128, n0:n0 + nt],
                            osb[b][:, j0:j0 + nt]
                        )
                elif j0 + nt == NT:
                    for b in range(B // 128):
                        nc.scalar.dma_start(
                            out_d[b * 128:(b + 1) * 128, g * NT:(g + 1) * NT],
                            osb[b][:]
                        )
    nc.compile()
    return nc


def _run_spmd(nc, in_maps):
    last_exc = None
    for _ in range(3):  # device occasionally needs one recovery execute
        try:
            return run_bass_kernel_spmd(nc, in_maps, core_ids=list(range(N_CORES)))
        except Exception as e:  # noqa: BLE001
            last_exc = e
    raise last_exc


def _shards_ok(res, f_hat, weight, w_inv, inv_scale):
    """Guard against flaky device executes (observed: a run can silently
    return decorrelated garbage). Check a 128-column probe block per core
    against the host; caller reruns on failure."""
    ok = True
    for i in range(N_CORES):
        n0 = i * N_SHARD
        w_hat_blk = weight[n0:n0 + 128] * (w_inv[n0:n0 + 128] / W_SCALE)
        ref = f_hat @ w_hat_blk.T                   # [B, 128] fp32
        got = res.results[i]["out"][:, :128].astype(np.float32) * inv_scale
        err = np.abs(got - ref).max()
        if not np.isfinite(err) or err > 0.05:
            print(f"kernel self-check: core {i} probe absmax {err:.3e} "
                  f"-> rerun", flush=True)
            ok = False
    return ok


def _swizzle_shard(wq):
    """[N_SHARD, K] row-major -> [128, N_CHUNKS, KC, NT] so each chunk's
    per-partition line (KC*NT bytes) is contiguous."""
    # buf[p, g, c, t] = wq[g*NT + t, c*128 + p]
    v = wq.reshape(N_CHUNKS, NT, KC, 128)
    return np.ascontiguousarray(v.transpose(3, 0, 2, 1))


def kernel(features, weight, threshold):
    features = np.asarray(features, dtype=np.float32)
    weight = np.asarray(weight, dtype=np.float32)

    f_norm = np.linalg.norm(features, axis=1, keepdims=True)
    f_hat = features / np.maximum(f_norm, EPS)
    if MODE == "e3s":
        fT = np.ascontiguousarray(f_hat.T * W_SCALE).astype(ml_dtypes.float8_e3m4)
    else:
        fT = np.ascontiguousarray(f_hat.T).astype(np.float16)  # [768, 256]

    w_norm = np.linalg.norm(weight, axis=1, keepdims=True)
    w_inv = (W_SCALE / np.maximum(w_norm, EPS)).astype(np.float32)
    wnp = np.float16 if MODE == "fp16x" else ml_dtypes.float8_e3m4

    shards = []
    for i in range(N_CORES):
        n0 = i * N_SHARD
        n1 = min(n0 + N_SHARD, N_FULL)
        s = np.zeros((N_SHARD, K), dtype=wnp)
        s[: n1 - n0] = (weight[n0:n1] * w_inv[n0:n1]).astype(wnp)
        shards.append(_swizzle_shard(s))

    key = ("nc", MODE)
    if key not in _CACHED:
        _CACHED[key] = _build_bass(MODE)
    nc = _CACHED[key]

    inv_scale = np.float32(1.0 / W_SCALE)
    if MODE == "e3s":
        inv_scale = np.float32(1.0 / (W_SCALE * W_SCALE))

    in_maps = [{"fT": fT, "wT": shards[i]} for i in range(N_CORES)]
    res = _run_spmd(nc, in_maps)
    for _ in range(3):
        if _shards_ok(res, f_hat, weight, w_inv, inv_scale):
            break
        res = _run_spmd(nc, in_maps)
    _CACHED["last_result"] = res
    out = np.empty((B, N_FULL + 1), dtype=np.float32)
    for i in range(N_CORES):
        n0 = i * N_SHARD
        n1 = min(n0 + N_SHARD, N_FULL)
        out[:, n0:n1] = res.results[i]["out"][:, : n1 - n0].astype(np.float32)
        out[:, n0:n1] *= inv_scale
    out[:, N_FULL] = np.float32(threshold)
    return out
